# revision 1
# baseline (speedup 1.0000x reference)
"""GCN (2x GCNConv + FC + sigmoid) on 8 Trainium2 NeuronCores.

Strategy (graph/data parallel, per sharding hint):
  - Nodes are partitioned across the 8 cores (with a load-balancing
    permutation so every 128-node chunk has a uniform padded edge-slot
    count); edges are assigned to the core owning their destination node.
  - GCN propagation is reformulated so each conv is:
        gather rows of a DRAM table (bf16, node-paired 256B rows) by edge
        source -> per-128-edge-tile one-hot segment-sum matmuls (bf16,
        fp32 PSUM accumulate) -> dense epilogue matmuls (W1/W2/Wfc) +
        activations.
    All degree normalization is folded into host-precomputed per-edge
    weights (graph-structure-only preprocessing) that scale the one-hot.
  - Launch 1 computes ys = dinv * (relu(conv1(x)) @ W2) node-blocks;
    the host reassembles the global ys table (free), launch 2 consumes it
    for conv2 + FC + sigmoid. No collectives needed.
"""
import sys

try:
    import concourse  # noqa: F401  (normally on PYTHONPATH via the axon site)
except ImportError:
    sys.path.insert(0, "/opt/trn_rl_repo")

from contextlib import ExitStack

import numpy as np
import ml_dtypes

import concourse.bass as bass
import concourse.tile as tile
from concourse import bacc, mybir
from concourse.bass_utils import run_bass_kernel_spmd

# ---- problem constants (hardcoded per spec) ----
N = 50000
NCORES = 8
BLOCK = N // NCORES           # 6250
P = 128
CHUNKS = (BLOCK + P - 1) // P  # 49
LAST_CAP = BLOCK - (CHUNKS - 1) * P  # 106
CPS = 2                        # chunks per gather slice (SWDGE ring capacity bound)

F32 = mybir.dt.float32
BF16 = mybir.dt.bfloat16
I16 = mybir.dt.int16
BF = ml_dtypes.bfloat16


# --------------------------------------------------------------------------
# host-side graph preprocessing (graph structure only -- no feature math)
# --------------------------------------------------------------------------
def _preprocess(edge_index):
    src = np.asarray(edge_index[0], dtype=np.int64)
    dst = np.asarray(edge_index[1], dtype=np.int64)

    loops = np.arange(N, dtype=np.int64)
    src2 = np.concatenate([src, loops])
    dst2 = np.concatenate([dst, loops])

    deg = np.bincount(dst2, minlength=N).astype(np.float64)
    dinv = (1.0 / np.sqrt(deg)).astype(np.float32)

    # per-node slot counts by source parity (the parity groups are padded
    # separately on-device, so the bin max over each parity drives padding)
    epar = (src2 & 1).astype(np.int64)
    cnt_par = np.zeros((N, 2), dtype=np.int64)
    np.add.at(cnt_par, (dst2, epar), 1)
    e_cnt, o_cnt = cnt_par[:, 0], cnt_par[:, 1]
    slots_per_node = e_cnt + o_cnt

    # parity-aware greedy binning into NCORES*CHUNKS bins (chunk = 128 nodes):
    # place big nodes first into the bin minimizing the resulting
    # max(even, odd) load (tie: total), respecting bin capacity.
    nbins = NCORES * CHUNKS
    cap = np.full(nbins, P, dtype=np.int64)
    cap[CHUNKS - 1::CHUNKS] = LAST_CAP
    order = np.argsort(-slots_per_node, kind="stable")
    fill = np.zeros(nbins, dtype=np.int64)
    even = np.zeros(nbins, dtype=np.int64)
    odd = np.zeros(nbins, dtype=np.int64)
    node_bin = np.empty(N, dtype=np.int64)
    node_pos = np.empty(N, dtype=np.int64)
    INF = np.int64(1 << 60)
    for v in order:
        e, o = e_cnt[v], o_cnt[v]
        score = np.maximum(even + e, odd + o) * (1 << 20) + (even + odd)
        score[fill >= cap] = INF
        b = int(np.argmin(score))
        node_bin[v] = b
        node_pos[v] = fill[b]
        fill[b] += 1
        even[b] += e
        odd[b] += o

    perm = -np.ones((NCORES, CHUNKS * P), dtype=np.int64)
    core_of = node_bin // CHUNKS
    chunk_of = node_bin % CHUNKS
    perm[core_of, chunk_of * P + node_pos] = np.arange(N)

    e_bin = node_bin[dst2]
    e_par = (src2 & 1).astype(np.int64)
    e_dstloc = node_pos[dst2]
    e_pair = src2 >> 1

    cnt = np.zeros((nbins, 2), dtype=np.int64)
    np.add.at(cnt, (e_bin, e_par), 1)
    T_E = int(np.ceil(cnt[:, 0].max() / P))
    T_O = int(np.ceil(cnt[:, 1].max() / P))
    T_C = T_E + T_O
    SLOTS = CHUNKS * T_C * P

    eorder = np.lexsort((e_par, e_bin))
    b_s = e_bin[eorder]
    p_s = e_par[eorder]
    key = b_s * 2 + p_s
    first = np.ones(len(eorder), dtype=bool)
    first[1:] = key[1:] != key[:-1]
    starts = np.flatnonzero(first)
    off_in_run = np.arange(len(eorder)) - starts[np.cumsum(first) - 1]

    core_s = b_s // CHUNKS
    chunk_s = b_s % CHUNKS
    slot = chunk_s * (T_C * P) + p_s * (T_E * P) + off_in_run

    pair_idx = np.zeros((NCORES, SLOTS), dtype=np.int16)
    dst_loc = -np.ones((NCORES, SLOTS), dtype=np.float32)
    w1 = np.zeros((NCORES, SLOTS), dtype=np.float32)
    w2 = np.zeros((NCORES, SLOTS), dtype=np.float32)
    ww1 = (dinv[src2] * dinv[dst2]).astype(np.float32)
    ww2 = dinv[dst2].astype(np.float32)
    pair_idx[core_s, slot] = e_pair[eorder].astype(np.int16)
    dst_loc[core_s, slot] = e_dstloc[eorder].astype(np.float32)
    w1[core_s, slot] = ww1[eorder]
    w2[core_s, slot] = ww2[eorder]

    dinv_local = np.ones((NCORES, CHUNKS * P), dtype=np.float32)
    m = perm >= 0
    dinv_local[m] = dinv[perm[m]]

    return dict(perm=perm, pair_idx=pair_idx, dst_loc=dst_loc, w1=w1, w2=w2,
                dinv_local=dinv_local, T_E=T_E, T_O=T_O, T_C=T_C, SLOTS=SLOTS)


# --------------------------------------------------------------------------
# device programs
# --------------------------------------------------------------------------
def _build(mode, T_E, T_O, chunk_limit=None, repeat=1, skip_gather=False):
    """mode: 'conv1' (x -> ys block) or 'conv2' (ys -> sigmoid out block)."""
    conv1 = mode == "conv1"
    T_C = T_E + T_O
    TT = CHUNKS * T_C              # total edge tiles per core
    SLOTS = TT * P
    TPS = CPS * T_C                # tiles per (full) slice
    FEAT = 27 if conv1 else 64
    nchunks = CHUNKS if chunk_limit is None else chunk_limit
    slices = [range(i, min(i + CPS, nchunks)) for i in range(0, nchunks, CPS)]
    MOFF = 64                      # parity column offset in paired table rows

    nc = bacc.Bacc("TRN2", target_bir_lowering=False, debug=False,
                   enable_asserts=False, num_devices=NCORES,
                   num_swdge_queues=4)
    table = nc.dram_tensor("table", [N // 2, 128], BF16, kind="ExternalInput")
    idx = nc.dram_tensor("idx", [128, SLOTS // 16], I16, kind="ExternalInput")
    ohmat = nc.dram_tensor("ohmat", [128, TT * 128], BF16, kind="ExternalInput")
    if conv1:
        w1 = nc.dram_tensor("w1", [27, 128], F32, kind="ExternalInput")
        b1 = nc.dram_tensor("b1", [128, 1], F32, kind="ExternalInput")
        w2 = nc.dram_tensor("w2", [128, 64], F32, kind="ExternalInput")
        dinv = nc.dram_tensor("dinv", [128, CHUNKS], F32, kind="ExternalInput")
        ys_out = nc.dram_tensor("ys_out", [CHUNKS * P, 64], F32,
                                kind="ExternalOutput")
    else:
        b2 = nc.dram_tensor("b2", [64, 1], F32, kind="ExternalInput")
        wfc = nc.dram_tensor("wfc", [64, 1], F32, kind="ExternalInput")
        bfc = nc.dram_tensor("bfc", [1, 1], F32, kind="ExternalInput")
        out = nc.dram_tensor("out", [1, CHUNKS * P], F32, kind="ExternalOutput")

    AF = mybir.ActivationFunctionType
    OP = mybir.AluOpType

    with tile.TileContext(nc) as tc, ExitStack() as ctx:
        cpool = ctx.enter_context(tc.tile_pool(name="const", bufs=1))
        mpool = ctx.enter_context(tc.tile_pool(name="msg", bufs=6))
        opool = ctx.enter_context(tc.tile_pool(name="oh", bufs=4))
        apool = ctx.enter_context(tc.tile_pool(name="agg", bufs=2, space="PSUM"))
        e1pool = ctx.enter_context(tc.tile_pool(name="ep1", bufs=2, space="PSUM"))
        tpool = ctx.enter_context(tc.tile_pool(name="tmp", bufs=2))
        if conv1:
            e2pool = ctx.enter_context(
                tc.tile_pool(name="ep2", bufs=2, space="PSUM"))

        idx_sb = cpool.tile([128, SLOTS // 16], I16)
        nc.sync.dma_start(idx_sb[:], idx.ap())
        if conv1:
            w1_sb = cpool.tile([27, 128], F32)
            nc.sync.dma_start(w1_sb[:], w1.ap())
            b1_sb = cpool.tile([128, 1], F32)
            nc.sync.dma_start(b1_sb[:], b1.ap())
            w2_sb = cpool.tile([128, 64], F32)
            nc.sync.dma_start(w2_sb[:], w2.ap())
            dinv_sb = cpool.tile([128, CHUNKS], F32)
            nc.sync.dma_start(dinv_sb[:], dinv.ap())
        else:
            b2_sb = cpool.tile([64, 1], F32)
            nc.sync.dma_start(b2_sb[:], b2.ap())
            wfc_sb = cpool.tile([64, 1], F32)
            nc.sync.dma_start(wfc_sb[:], wfc.ap())
            bfc_sb = cpool.tile([1, 1], F32)
            nc.sync.dma_start(bfc_sb[:], bfc.ap())

        def emit_body():
          for sl_i, chunk_range in enumerate(slices):
            n_sl_tiles = len(chunk_range) * T_C
            sl_slots = n_sl_tiles * P
            t0_tile = chunk_range[0] * T_C
            msg = mpool.tile([128, TPS * 128], BF16)
            if skip_gather:
                nc.vector.memset(msg[:, 0:2], 0.0)
            if not skip_gather:
                msg3 = msg[:, :n_sl_tiles * 128].rearrange(
                    "p (t e) -> p t e", e=128)
                nc.gpsimd.dma_gather(
                    msg3, table.ap(),
                    idx_sb[:, t0_tile * 8:(t0_tile + n_sl_tiles) * 8],
                    sl_slots, sl_slots, 128, single_packet=False,
                    queue_num=sl_i % 4)
            ohsl = opool.tile([128, TPS * 128], BF16)
            nc.sync.dma_start(
                ohsl[:, :n_sl_tiles * 128],
                ohmat.ap()[:, t0_tile * 128:(t0_tile + n_sl_tiles) * 128])

            for ci, c in enumerate(chunk_range):
                agg = apool.tile([32 if conv1 else 64, 128], F32)
                for t in range(T_C):
                    g = ci * T_C + t
                    off = 0 if t < T_E else MOFF
                    nc.tensor.matmul(
                        agg[0:FEAT, :],
                        lhsT=msg[:, g * 128 + off: g * 128 + off + FEAT],
                        rhs=ohsl[:, g * 128:(g + 1) * 128],
                        start=(t == 0), stop=(t == T_C - 1))

                if conv1:
                    aggsb = tpool.tile([32, 128], F32, tag="aggsb")
                    nc.scalar.activation(aggsb[0:27, :], agg[0:27, :], AF.Copy)
                    h1p = e1pool.tile([128, 128], F32)
                    nc.tensor.matmul(h1p[:], lhsT=w1_sb[:], rhs=aggsb[0:27, :],
                                     start=True, stop=True)
                    h1sb = tpool.tile([128, 128], F32, tag="h1sb")
                    nc.scalar.activation(h1sb[:], h1p[:], AF.Relu,
                                         bias=b1_sb[:])
                    ysp = e2pool.tile([128, 64], F32)
                    nc.tensor.matmul(ysp[:], lhsT=h1sb[:], rhs=w2_sb[:],
                                     start=True, stop=True)
                    yssb = tpool.tile([128, 64], F32, tag="yssb")
                    nc.vector.tensor_scalar(yssb[:], ysp[:],
                                            dinv_sb[:, c:c + 1], None,
                                            op0=OP.mult)
                    nc.sync.dma_start(ys_out.ap()[c * P:(c + 1) * P, :],
                                      yssb[:])
                else:
                    h2sb = tpool.tile([64, 128], F32, tag="h2sb")
                    nc.scalar.activation(h2sb[:], agg[0:64, :], AF.Relu,
                                         bias=b2_sb[:])
                    lgp = e1pool.tile([1, 128], F32)
                    nc.tensor.matmul(lgp[0:1, :], lhsT=wfc_sb[:], rhs=h2sb[:],
                                     start=True, stop=True)
                    osb = tpool.tile([1, 128], F32, tag="osb")
                    nc.scalar.activation(osb[0:1, :], lgp[0:1, :], AF.Sigmoid,
                                         bias=bfc_sb[0:1, :])
                    nc.sync.dma_start(out.ap()[0:1, c * P:(c + 1) * P],
                                      osb[0:1, :])

        if repeat == 1:
            emit_body()
        else:
            with tc.For_i(0, repeat, 1):
                emit_body()
    nc.compile()
    return nc


_PROG_CACHE = {}


def _programs(T_E, T_O):
    key = (T_E, T_O)
    if key not in _PROG_CACHE:
        _PROG_CACHE[key] = (_build("conv1", T_E, T_O),
                            _build("conv2", T_E, T_O))
    return _PROG_CACHE[key]


# --------------------------------------------------------------------------
# host orchestration
# --------------------------------------------------------------------------
_LAST_EXEC_NS = None


def _wrap_idx(pair_idx):
    s = pair_idx.shape[0]
    return np.ascontiguousarray(np.tile(pair_idx.reshape(s // 16, 16).T, (8, 1)))


def _tile_major(arr):
    # [SLOTS] -> [128, SLOTS//128] with [p, t] = arr[t*128 + p]
    return np.ascontiguousarray(arr.reshape(-1, 128).T)


def _ohmat(dst_loc, w):
    """Host-built scaled one-hot tiles: [128, TT*128] bf16 with
    ohmat[p, gt*128 + dst_loc[slot]] = w[slot] for slot = gt*128 + p."""
    slots = dst_loc.shape[0]
    tt = slots // 128
    oh = np.zeros((128, tt * 128), dtype=BF)
    sl = np.arange(slots)
    valid = dst_loc >= 0
    p = sl[valid] % 128
    col = (sl[valid] // 128) * 128 + dst_loc[valid].astype(np.int64)
    oh[p, col] = w[valid].astype(BF)
    return oh


def kernel(x, edge_index, W1, b1, W2, b2, Wfc, bfc):
    x = np.asarray(x, dtype=np.float32)
    W1 = np.asarray(W1, dtype=np.float32)
    b1 = np.asarray(b1, dtype=np.float32)
    W2 = np.asarray(W2, dtype=np.float32)
    b2 = np.asarray(b2, dtype=np.float32)
    Wfc = np.asarray(Wfc, dtype=np.float32)
    bfc = np.asarray(bfc, dtype=np.float32)

    pp = _preprocess(np.asarray(edge_index))
    T_E, T_O, T_C = pp["T_E"], pp["T_O"], pp["T_C"]
    nc1, nc2 = _programs(T_E, T_O)

    # conv1 paired table: [25000, 128] bf16; even node at cols 0:27, odd at 64:91
    t1 = np.zeros((N // 2, 128), dtype=BF)
    t1[:, 0:27] = x[0::2].astype(BF)
    t1[:, 64:64 + 27] = x[1::2].astype(BF)

    in_maps1 = []
    for core in range(NCORES):
        in_maps1.append(dict(
            table=t1,
            idx=_wrap_idx(pp["pair_idx"][core]),
            ohmat=_ohmat(pp["dst_loc"][core], pp["w1"][core]),
            w1=W1,
            b1=np.ascontiguousarray(b1[:, None]),
            w2=W2,
            dinv=_tile_major(pp["dinv_local"][core]),
        ))
    res1 = run_bass_kernel_spmd(nc1, in_maps1, core_ids=list(range(NCORES)))

    ys_g = np.zeros((N, 64), dtype=np.float32)
    for core in range(NCORES):
        pr = pp["perm"][core]
        m = pr >= 0
        ys_g[pr[m]] = res1.results[core]["ys_out"][m]

    t2 = np.zeros((N // 2, 128), dtype=BF)
    t2[:, 0:64] = ys_g[0::2].astype(BF)
    t2[:, 64:128] = ys_g[1::2].astype(BF)

    in_maps2 = []
    for core in range(NCORES):
        in_maps2.append(dict(
            table=t2,
            idx=_wrap_idx(pp["pair_idx"][core]),
            ohmat=_ohmat(pp["dst_loc"][core], pp["w2"][core]),
            b2=np.ascontiguousarray(b2[:, None]),
            wfc=Wfc,
            bfc=bfc.reshape(1, 1),
        ))
    res2 = run_bass_kernel_spmd(nc2, in_maps2, core_ids=list(range(NCORES)))

    out_g = np.zeros((N,), dtype=np.float32)
    for core in range(NCORES):
        pr = pp["perm"][core]
        m = pr >= 0
        out_g[pr[m]] = res2.results[core]["out"][0][m]

    global _LAST_EXEC_NS
    e1, e2 = res1.exec_time_ns, res2.exec_time_ns
    _LAST_EXEC_NS = None if e1 is None and e2 is None else (e1 or 0) + (e2 or 0)
    return out_g[:, None]

